# revision 23
# baseline (speedup 1.0000x reference)
"""BitNet attention (GQA + RoPE) on 8 Trainium2 NeuronCores.

Tensor-parallel over heads: core c owns q-heads [4c, 4c+4), kv-head c.
Each core computes q/k/v projections (ternary BitNet weights), RoPE,
attention for its heads, and a row-parallel partial of the Wo
projection; the host sums the 8 bf16 partials in f32.

v6 design (measured HW facts: PE = 1 cyc/moving-row for bf16 AND f32r
at 2.4 GHz after a 3us ramp; Scalar EXP = free-size x ~1.07 ns and is
the only engine with EXP -> 140 us/core of softmax exps is the
irreducible stage):
  - bf16 everywhere: ternary weights are exact in bf16; x/outputs in
    bf16 cost ~0.2% rms each. Halves all HBM traffic.
  - x^T is SBUF-resident (8.4 MB) so KV projections, then per-block Q
    projections, stream without re-loading.
  - The attention chunk loop is software-pipelined: per key-chunk i the
    PE does scores(i) [dual-tile bf16 pair], AV(i-1), and two "slot"
    matmuls -- Q(j+1) chunks at p==0, Wo(j-1) pieces at p==1 -- so the
    Scalar engine stays saturated while Q/Wo ride along for free.
  - Softmax denominators: V col 64 = exp(mask), so AV row 64 is the
    denominator; reciprocal_approx_accurate (2.8x faster than DVE
    reciprocal) + DRAM-broadcast + DVE mul normalizes aoT per half.
  - PSUM budget exactly 8 banks: scores [128,1024]x2 (4) + AV pair (2)
    + shared Q/Wo work pair (2).

The attention mask is folded into the V tile: attn = exp(s*qk + m) =
exp(m)*exp(s*qk), so V rows and the denominator-ones column are
pre-scaled by exp(mask) and the EXP activation needs no bias.
"""

import sys

if "/opt/trn_rl_repo" not in sys.path:
    sys.path.insert(0, "/opt/trn_rl_repo")

import numpy as np
from ml_dtypes import bfloat16

import concourse.bass as bass
from concourse import bacc, mybir
from concourse.bass import ts
from concourse.bass_utils import run_bass_kernel_spmd
from concourse.masks import make_identity
from concourse.tile import TileContext

F32 = mybir.dt.float32
BF16 = mybir.dt.bfloat16

S = 2048
H = 2048
N_HEADS = 32
N_KV = 8
D = 64
NCORES = 8
HPC = N_HEADS // NCORES  # 4 q heads per core
OC = HPC * D  # 256 output dims per core
NB = S // 512  # 4 s-blocks of 512
HC = H // 128  # 16 hidden chunks

LAST_EXEC_NS = None
LAST_TRACE = None
_CACHE = {}


def _ternarize(w):
    w = np.asarray(w, np.float32)
    s = (np.abs(w).mean() + np.float32(1e-6)).astype(np.float32)
    t = np.round(np.clip(w / s, np.float32(-1.0), np.float32(1.0))).astype(np.float32)
    return t, float(s)


def _build_program(s_qk):
    nc = bacc.Bacc("TRN2", target_bir_lowering=False, debug=False, num_devices=NCORES)

    xt = nc.dram_tensor("xt", [NB, 128, HC, 512], BF16, kind="ExternalInput")
    wq = nc.dram_tensor("wq_t", [128, HC, OC], BF16, kind="ExternalInput")
    wkv = nc.dram_tensor("wkv_t", [128, HC, 128], BF16, kind="ExternalInput")
    wo = nc.dram_tensor("wo_t", [128, 2, H], BF16, kind="ExternalInput")
    cos_d = nc.dram_tensor("cos_t", [128, S], F32, kind="ExternalInput")
    sin_d = nc.dram_tensor("sin_t", [128, S], F32, kind="ExternalInput")
    emv_d = nc.dram_tensor("emv_t", [128, HC], F32, kind="ExternalInput")
    em_d = nc.dram_tensor("em_t", [128, HC], BF16, kind="ExternalInput")
    outp = nc.dram_tensor("outp", [S, H], BF16, kind="ExternalOutput")

    EXP = mybir.ActivationFunctionType.Exp
    MUL = mybir.AluOpType.mult
    ADD = mybir.AluOpType.add

    with TileContext(nc) as tc:
        with tc.tile_pool(name="persist", bufs=1) as persist:
            qT = persist.tile([128, 2, S], BF16)
            kTd = persist.tile([128, S], BF16)
            V = persist.tile([128, HC, 65], BF16)
            aoT = persist.tile([128, 2, S], BF16)
            xt_sb = persist.tile([128, NB, HC, 512], BF16)
            wq_sb = persist.tile([128, HC, OC], BF16)
            wkv_sb = persist.tile([128, HC, 128], BF16)
            wo_sb = persist.tile([128, 2, H], BF16)
            cos_sb = persist.tile([128, S], F32)
            sin_sb = persist.tile([128, S], F32)
            emv_sb = persist.tile([128, HC], F32)
            vT = persist.tile([64, S], BF16)
            ident = persist.tile([64, 64], BF16)

            # --- input DMAs, critical-path first ---
            nc.sync.dma_start(wkv_sb[:], wkv[:])
            for c4 in range(4):
                nc.sync.dma_start(
                    xt_sb[:, 0, ts(c4, 4), :], xt[0, :, ts(c4, 4), :]
                )
            nc.gpsimd.dma_start(wq_sb[:], wq[:])
            nc.gpsimd.dma_start(cos_sb[:, 0:1024], cos_d[:, 0:1024])
            nc.gpsimd.dma_start(sin_sb[:, 0:1024], sin_d[:, 0:1024])
            nc.gpsimd.dma_start(emv_sb[:], emv_d[:])
            for b in range(1, NB):
                for c2 in range(2):
                    nc.sync.dma_start(
                        xt_sb[:, b, ts(c2, 8), :], xt[b, :, ts(c2, 8), :]
                    )
            nc.gpsimd.dma_start(cos_sb[:, 1024:2048], cos_d[:, 1024:2048])
            nc.gpsimd.dma_start(sin_sb[:, 1024:2048], sin_d[:, 1024:2048])
            nc.gpsimd.dma_start(wo_sb[:], wo[:])
            make_identity(nc, ident[:])
            for i in range(HC):
                nc.gpsimd.dma_start(V[:, i, 64:65], em_d[:, i : i + 1])

            # ---- Phase A: KV projections, K-RoPE, V transpose ----
            with (
                tc.tile_pool(name="kvps", bufs=2, space="PSUM") as kvps,
                tc.tile_pool(name="vtps", bufs=2, space="PSUM") as vtps,
                tc.tile_pool(name="q0ps", bufs=1, space="PSUM") as q0ps,
                tc.tile_pool(name="ph1t", bufs=3) as ph1t,
            ):
                # warm the EXP activation table while DMAs stream
                wrm = ph1t.tile([1, 2], F32, tag="wrm")
                nc.scalar.activation(wrm[:], ident[0:1, 0:2], EXP)
                # pre-warm the PE as soon as wkv lands: the clock needs ~3us
                # of gap-free matmuls to reach 2.4 GHz and the whole lead-in
                # otherwise runs at half clock
                for w in range(10):
                    wup = kvps.tile([128, 512], F32, tag="kv", name=f"wup_{w}")
                    nc.tensor.matmul(
                        wup[:], wkv_sb[:, w, :], wkv_sb[:, ts(w % 4, 4), :],
                        start=True, stop=True,
                    )
                for b in range(NB):
                    sb = ts(b, 512)
                    pkv = kvps.tile([128, 512], F32, tag="kv", name=f"pkv_{b}")
                    for c in range(HC):
                        nc.tensor.matmul(
                            pkv[:], wkv_sb[:, c, :], xt_sb[:, b, c, :],
                            start=c == 0, stop=c == HC - 1,
                        )
                    # K path (DVE): RoPE, duplicated on both halves
                    rotk = ph1t.tile([64, 512], F32, tag="rotk")
                    nc.vector.tensor_copy(rotk[0:32, :], pkv[32:64, :])
                    nc.vector.tensor_copy(rotk[32:64, :], pkv[0:32, :])
                    kc = ph1t.tile([64, 512], F32, tag="kc")
                    nc.vector.tensor_tensor(kc[:], pkv[0:64, :], cos_sb[0:64, sb], MUL)
                    ks = ph1t.tile([64, 512], F32, tag="ks")
                    nc.vector.tensor_tensor(ks[:], rotk[:], sin_sb[0:64, sb], MUL)
                    nc.vector.tensor_tensor(kTd[0:64, sb], kc[:], ks[:], ADD)
                    nc.vector.tensor_tensor(kTd[64:128, sb], kc[:], ks[:], ADD)
                    # V row extract on the idle Scalar engine
                    nc.scalar.copy(vT[:, sb], pkv[64:128, :])
                    if b == 0:
                        # Q(0) rides right behind KV(0): xt(0) is resident and
                        # later xt blocks are still in flight on DMA
                        pq0 = q0ps.tile([128, 512], F32, tag="q0a", name="pq0_0")
                        pq1 = q0ps.tile([128, 512], F32, tag="q0b", name="pq1_0")
                        for c in range(HC):
                            nc.tensor.matmul(
                                pq0[:], wq_sb[:, c, 0:128], xt_sb[:, 0, c, :],
                                start=c == 0, stop=c == HC - 1,
                            )
                            nc.tensor.matmul(
                                pq1[:], wq_sb[:, c, 128:256], xt_sb[:, 0, c, :],
                                start=c == 0, stop=c == HC - 1,
                            )
                # V transposes after the KV matmul stream (no mid-stream PE
                # stalls); scales on Scalar
                for i in range(HC):
                    pt = vtps.tile([128, 64], BF16, tag="vt", name=f"pt_{i}")
                    nc.tensor.transpose(pt[:], vT[:, ts(i, 128)], ident[:])
                    nc.scalar.mul(V[:, i, 0:64], pt[:], emv_sb[:, i : i + 1])
                # RoPE for Q(0)
                for p_, pq in ((0, pq0), (1, pq1)):
                    rot = ph1t.tile([128, 512], F32, tag="rot")
                    nc.vector.tensor_copy(rot[0:32, :], pq[32:64, :])
                    nc.vector.tensor_copy(rot[32:64, :], pq[0:32, :])
                    nc.vector.tensor_copy(rot[64:96, :], pq[96:128, :])
                    nc.vector.tensor_copy(rot[96:128, :], pq[64:96, :])
                    qc = ph1t.tile([128, 512], F32, tag="qc")
                    nc.vector.tensor_tensor(qc[:], pq[:], cos_sb[:, 0:512], MUL)
                    qs = ph1t.tile([128, 512], F32, tag="qs")
                    nc.vector.tensor_tensor(qs[:], rot[:], sin_sb[:, 0:512], MUL)
                    nc.vector.tensor_tensor(qT[:, p_, 0:512], qc[:], qs[:], ADD)

            # ---- main loop: Q(0), then per j: attention with Q(j+1)/Wo(j-1)
            # matmuls interleaved into the chunk slots ----
            with (
                tc.tile_pool(name="expp", bufs=4) as expp,
                tc.tile_pool(name="ropet", bufs=3) as ropet,
                tc.tile_pool(name="avp", bufs=2) as avp,
                tc.tile_pool(name="dnp", bufs=2) as dnp,
                tc.tile_pool(name="cbp", bufs=4) as cbp,
                tc.tile_pool(name="obp", bufs=3) as obp,
                tc.tile_pool(name="oap", bufs=4) as oap,
                tc.tile_pool(name="csd", bufs=4, space="DRAM") as csd,
                tc.tile_pool(name="pssc", bufs=2, space="PSUM") as pssc,
                tc.tile_pool(name="psav", bufs=1, space="PSUM") as psav,
                tc.tile_pool(name="pwork", bufs=1, space="PSUM") as pwork,
            ):

                def q_rope(jq_, pq0_, pq1_):
                    sb = ts(jq_, 512)
                    for p_, pq in ((0, pq0_), (1, pq1_)):
                        rot = ropet.tile([128, 512], F32, tag="rot")
                        nc.vector.tensor_copy(rot[0:32, :], pq[32:64, :])
                        nc.vector.tensor_copy(rot[32:64, :], pq[0:32, :])
                        nc.vector.tensor_copy(rot[64:96, :], pq[96:128, :])
                        nc.vector.tensor_copy(rot[96:128, :], pq[64:96, :])
                        qc = ropet.tile([128, 512], F32, tag="qc")
                        nc.vector.tensor_tensor(qc[:], pq[:], cos_sb[:, sb], MUL)
                        qs = ropet.tile([128, 512], F32, tag="qs")
                        nc.vector.tensor_tensor(qs[:], rot[:], sin_sb[:, sb], MUL)
                        nc.vector.tensor_tensor(qT[:, p_, sb], qc[:], qs[:], ADD)

                ob_state = {}
                oa_tiles = {}

                def emit_wo_piece(jwo, i):
                    # i in 0..15 -> (local q-chunk, hidden block)
                    jq = 4 * jwo + i // 4
                    hb = i % 4
                    tag = "wa" if i % 2 == 0 else "wb"
                    po = pwork.tile([128, 512], F32, tag=tag, name=f"po_{jq}_{hb}")
                    nc.tensor.matmul(
                        po[:], aoT[:, 0, ts(jq, 128)], wo_sb[:, 0, ts(hb, 512)],
                        start=True, stop=False,
                    )
                    nc.tensor.matmul(
                        po[:], aoT[:, 1, ts(jq, 128)], wo_sb[:, 1, ts(hb, 512)],
                        start=False, stop=True,
                    )
                    if hb == 0:
                        ob_state["ob"] = obp.tile(
                            [128, H], BF16, tag="ob", name=f"ob_{jq}"
                        )
                    ob = ob_state["ob"]
                    nc.vector.tensor_copy(ob[:, ts(hb, 512)], po[:])
                    if hb == 3:
                        eng = nc.sync if jq % 2 == 0 else nc.gpsimd
                        eng.dma_start(outp[ts(jq, 128), :], ob[:])

                def emit_wo_half_a(i):
                    # Wo(NB-1) first-contract-half: only needs aoT[:,0] (p0,
                    # normalized long ago); staged so the second half +
                    # epilogue is all that remains after the last normalize.
                    jq = 4 * (NB - 1) + i // 4
                    hb = i % 4
                    tag = "wa" if i % 2 == 0 else "wb"
                    po = pwork.tile([128, 512], F32, tag=tag, name=f"poa_{jq}_{hb}")
                    nc.tensor.matmul(
                        po[:], aoT[:, 0, ts(jq, 128)], wo_sb[:, 0, ts(hb, 512)],
                        start=True, stop=True,
                    )
                    if hb == 0:
                        oa_tiles[jq] = oap.tile(
                            [128, H], BF16, tag="oa", name=f"oa_{jq}"
                        )
                    nc.vector.tensor_copy(oa_tiles[jq][:, ts(hb, 512)], po[:])

                for j in range(NB):
                    jb = ts(j, 512)
                    for p in range(2):
                        do_q = p == 0 and j + 1 < NB
                        do_wo = (p == 1 and 1 <= j < NB - 1) or (
                            p == 0 and j == NB - 1
                        )
                        do_woa = p == 1 and j == NB - 1
                        if do_q:
                            npq0 = pwork.tile(
                                [128, 512], F32, tag="wa", name=f"pq0_{j + 1}"
                            )
                            npq1 = pwork.tile(
                                [128, 512], F32, tag="wb", name=f"pq1_{j + 1}"
                            )
                        pA = psav.tile([65, 512], F32, tag="avA", name=f"pA_{j}_{p}")
                        pB = psav.tile([65, 512], F32, tag="avB", name=f"pB_{j}_{p}")

                        def av_and_slot(i, e2):
                            st, sp = i == 0, i == HC - 1
                            nc.tensor.matmul(
                                pA[:], V[:, i, :], e2[:, 0:512], start=st, stop=sp
                            )
                            nc.tensor.matmul(
                                pB[:], V[:, i, :], e2[:, 512:1024], start=st, stop=sp
                            )
                            if do_q:
                                nc.tensor.matmul(
                                    npq0[:], wq_sb[:, i, 0:128],
                                    xt_sb[:, j + 1, i, :], start=st, stop=sp,
                                )
                                nc.tensor.matmul(
                                    npq1[:], wq_sb[:, i, 128:256],
                                    xt_sb[:, j + 1, i, :], start=st, stop=sp,
                                )
                            elif do_wo:
                                emit_wo_piece(j - 1, i)
                            elif do_woa:
                                emit_wo_half_a(i)

                        pend = []
                        for i in range(HC):
                            psAB = pssc.tile(
                                [128, 1024], F32, tag="sAB", name=f"sAB_{j}_{p}_{i}"
                            )
                            nc.tensor.matmul(
                                psAB[:, 0:512], kTd[0:64, ts(i, 128)],
                                qT[0:64, p, jb], start=True, stop=True,
                            )
                            nc.tensor.matmul(
                                psAB[:, 512:1024], kTd[64:128, ts(i, 128)],
                                qT[64:128, p, jb], start=True, stop=True,
                            )
                            e2 = expp.tile(
                                [128, 1024], BF16, tag="e2", name=f"e2_{j}_{p}_{i}"
                            )
                            nc.scalar.activation(e2[:], psAB[:], EXP, scale=s_qk)
                            pend.append((i, e2))
                            if len(pend) > 2:
                                av_and_slot(*pend.pop(0))
                        for it in pend:
                            av_and_slot(*it)

                        # stage AV out of PSUM, build denominators, normalize
                        av = avp.tile([128, 512], F32, tag="av", name=f"av_{j}_{p}")
                        nc.vector.tensor_copy(av[0:64, :], pA[0:64, :])
                        nc.vector.tensor_copy(av[64:128, :], pB[0:64, :])
                        dn = dnp.tile([33, 512], F32, tag="dn")
                        nc.vector.tensor_copy(dn[0:1, :], pA[64:65, :])
                        nc.vector.tensor_copy(dn[32:33, :], pB[64:65, :])
                        rcs = dnp.tile([33, 512], F32, tag="rcs")
                        scr = dnp.tile([33, 512], F32, tag="scr")
                        nc.vector.reciprocal_approx_accurate(rcs[:], dn[:], scr[:])
                        cs_dram = csd.tile([2, 1, 512], F32, tag="csd")
                        nc.sync.dma_start(cs_dram[0], rcs[0:1, :])
                        nc.sync.dma_start(cs_dram[1], rcs[32:33, :])
                        cb = cbp.tile([128, 512], F32, tag="cb", name=f"cb_{j}_{p}")
                        for h in range(2):
                            nc.sync.dma_start(
                                cb[h * 64 : (h + 1) * 64, :],
                                cs_dram[h].to_broadcast((64, 512)),
                            )
                        nc.vector.tensor_tensor(aoT[:, p, jb], av[:], cb[:], MUL)

                        if do_q:
                            q_rope(j + 1, npq0, npq1)

                # Wo(NB-1) epilogue: second contract half + add the staged
                # first half, then the last output DMAs. Warm matmuls first so
                # the PE re-ramps while the last normalize chain drains.
                for w in range(8):
                    wrmt = pwork.tile(
                        [128, 512], F32, tag="wa" if w % 2 == 0 else "wb",
                        name=f"wrmt_{w}",
                    )
                    nc.tensor.matmul(
                        wrmt[:], wkv_sb[:, w, :], xt_sb[:, NB - 1, w, :],
                        start=True, stop=True,
                    )
                for i in range(HC):
                    jq = 4 * (NB - 1) + i // 4
                    hb = i % 4
                    tag = "wa" if i % 2 == 0 else "wb"
                    po = pwork.tile([128, 512], F32, tag=tag, name=f"pob_{jq}_{hb}")
                    nc.tensor.matmul(
                        po[:], aoT[:, 1, ts(jq, 128)], wo_sb[:, 1, ts(hb, 512)],
                        start=True, stop=True,
                    )
                    if hb == 0:
                        ob_state["ob"] = obp.tile(
                            [128, H], BF16, tag="ob", name=f"obf_{jq}"
                        )
                    ob = ob_state["ob"]
                    nc.vector.tensor_tensor(
                        ob[:, ts(hb, 512)], po[:], oa_tiles[jq][:, ts(hb, 512)], ADD
                    )
                    if hb == 3:
                        eng = nc.sync if jq % 2 == 0 else nc.gpsimd
                        eng.dma_start(outp[ts(jq, 128), :], ob[:])

    nc.compile()
    return nc


def kernel(
    hidden_states,
    attention_mask,
    position_ids,
    wq,
    wk,
    wv,
    wo,
    _trace=False,
):
    global LAST_EXEC_NS, LAST_TRACE
    x = np.asarray(hidden_states, np.float32)[0]  # [S, H]
    mask = np.asarray(attention_mask, np.float32)[0]  # [S]
    pos = np.asarray(position_ids)[0].astype(np.float32)  # [S]

    wq_t, s_q = _ternarize(wq)
    wk_t, s_k = _ternarize(wk)
    wv_t, s_v = _ternarize(wv)
    wo_t, s_o = _ternarize(wo)
    s_qk = float(np.float32(s_q) * np.float32(s_k) / np.float32(8.0))
    s_vo = np.float32(s_v) * np.float32(s_o)

    key = ("v6", s_qk)
    if key not in _CACHE:
        _CACHE.clear()
        _CACHE[key] = _build_program(s_qk)
    nc = _CACHE[key]

    # shared inputs
    xt_host = np.ascontiguousarray(
        x.T.reshape(HC, 128, NB, 512).transpose(2, 1, 0, 3)
    ).astype(bfloat16)
    inv = (
        1.0 / (10000.0 ** (np.arange(0, D, 2, dtype=np.float32) / np.float32(D)))
    ).astype(np.float32)
    fr = pos[:, None] * inv[None, :]  # [S, 32]
    emb = np.concatenate([fr, fr], axis=1)  # [S, 64]
    cos64 = np.cos(emb).astype(np.float32)
    sin64 = np.sin(emb).astype(np.float32)
    sin64[:, : D // 2] *= -1.0
    cos128 = np.ascontiguousarray(np.vstack([cos64.T, cos64.T]))  # [128, S]
    sin128 = np.ascontiguousarray(np.vstack([sin64.T, sin64.T]))
    expmask = np.exp(mask).astype(np.float32)  # [S]
    em_r = np.ascontiguousarray(expmask.reshape(HC, 128).T).astype(bfloat16)
    emv_r = np.ascontiguousarray(
        (expmask * s_vo).reshape(HC, 128).T
    ).astype(np.float32)

    in_maps = []
    for c in range(NCORES):
        wq_c = np.ascontiguousarray(
            wq_t[c * OC : (c + 1) * OC, :].T.reshape(HC, 128, OC).transpose(1, 0, 2)
        ).astype(bfloat16)
        wk_c = wk_t[c * D : (c + 1) * D, :].T  # [H, 64]
        wv_c = wv_t[c * D : (c + 1) * D, :].T
        wkv_c = np.ascontiguousarray(
            np.concatenate([wk_c, wv_c], axis=1).reshape(HC, 128, 128).transpose(1, 0, 2)
        ).astype(bfloat16)
        wo_c = np.ascontiguousarray(
            wo_t[:, c * OC : (c + 1) * OC].T.reshape(2, 128, H).transpose(1, 0, 2)
        ).astype(bfloat16)
        in_maps.append(
            {
                "xt": xt_host,
                "wq_t": wq_c,
                "wkv_t": wkv_c,
                "wo_t": wo_c,
                "cos_t": cos128,
                "sin_t": sin128,
                "emv_t": emv_r,
                "em_t": em_r,
            }
        )

    res = run_bass_kernel_spmd(
        nc, in_maps, core_ids=list(range(NCORES)), trace=bool(_trace)
    )
    LAST_EXEC_NS = res.exec_time_ns
    LAST_TRACE = res.instructions_and_trace[1] if res.instructions_and_trace else None

    out = res.results[0]["outp"].astype(np.float32)
    for c in range(1, NCORES):
        out = out + res.results[c]["outp"].astype(np.float32)
    return out.reshape(1, S, H).astype(np.float32)


# revision 25
# speedup vs baseline: 1.0196x; 1.0196x over previous
"""BitNet attention (GQA + RoPE) on 8 Trainium2 NeuronCores.

Tensor-parallel over heads: core c owns q-heads [4c, 4c+4), kv-head c.
Each core computes q/k/v projections (ternary BitNet weights), RoPE,
attention for its heads, and a row-parallel partial of the Wo
projection; the host sums the 8 bf16 partials in f32.

v6 design (measured HW facts: PE = 1 cyc/moving-row for bf16 AND f32r
at 2.4 GHz after a 3us ramp; Scalar EXP = free-size x ~1.07 ns and is
the only engine with EXP -> 140 us/core of softmax exps is the
irreducible stage):
  - bf16 everywhere: ternary weights are exact in bf16; x/outputs in
    bf16 cost ~0.2% rms each. Halves all HBM traffic.
  - x^T is SBUF-resident (8.4 MB) so KV projections, then per-block Q
    projections, stream without re-loading.
  - The attention chunk loop is software-pipelined: per key-chunk i the
    PE does scores(i) [dual-tile bf16 pair], AV(i-1), and two "slot"
    matmuls -- Q(j+1) chunks at p==0, Wo(j-1) pieces at p==1 -- so the
    Scalar engine stays saturated while Q/Wo ride along for free.
  - Softmax denominators: V col 64 = exp(mask), so AV row 64 is the
    denominator; reciprocal_approx_accurate (2.8x faster than DVE
    reciprocal) + DRAM-broadcast + DVE mul normalizes aoT per half.
  - PSUM budget exactly 8 banks: scores [128,1024]x2 (4) + AV pair (2)
    + shared Q/Wo work pair (2).

The attention mask is folded into the V tile: attn = exp(s*qk + m) =
exp(m)*exp(s*qk), so V rows and the denominator-ones column are
pre-scaled by exp(mask) and the EXP activation needs no bias.
"""

import sys

if "/opt/trn_rl_repo" not in sys.path:
    sys.path.insert(0, "/opt/trn_rl_repo")

import numpy as np
from ml_dtypes import bfloat16

import concourse.bass as bass
from concourse import bacc, mybir
from concourse.bass import ts
from concourse.bass_utils import run_bass_kernel_spmd
from concourse.masks import make_identity
from concourse.tile import TileContext

F32 = mybir.dt.float32
BF16 = mybir.dt.bfloat16

S = 2048
H = 2048
N_HEADS = 32
N_KV = 8
D = 64
NCORES = 8
HPC = N_HEADS // NCORES  # 4 q heads per core
OC = HPC * D  # 256 output dims per core
NB = S // 512  # 4 s-blocks of 512
HC = H // 128  # 16 hidden chunks

LAST_EXEC_NS = None
LAST_TRACE = None
_CACHE = {}


def _ternarize(w):
    w = np.asarray(w, np.float32)
    s = (np.abs(w).mean() + np.float32(1e-6)).astype(np.float32)
    t = np.round(np.clip(w / s, np.float32(-1.0), np.float32(1.0))).astype(np.float32)
    return t, float(s)


def _build_program(s_qk):
    nc = bacc.Bacc("TRN2", target_bir_lowering=False, debug=False, num_devices=NCORES)

    xt = nc.dram_tensor("xt", [NB, 128, HC, 512], BF16, kind="ExternalInput")
    wq = nc.dram_tensor("wq_t", [128, HC, OC], BF16, kind="ExternalInput")
    wkv = nc.dram_tensor("wkv_t", [128, HC, 128], BF16, kind="ExternalInput")
    wo = nc.dram_tensor("wo_t", [128, 2, H], BF16, kind="ExternalInput")
    cos_d = nc.dram_tensor("cos_t", [128, S], F32, kind="ExternalInput")
    sin_d = nc.dram_tensor("sin_t", [128, S], F32, kind="ExternalInput")
    emv_d = nc.dram_tensor("emv_t", [128, HC], F32, kind="ExternalInput")
    em_d = nc.dram_tensor("em_t", [128, HC], BF16, kind="ExternalInput")
    outp = nc.dram_tensor("outp", [S, H], BF16, kind="ExternalOutput")

    EXP = mybir.ActivationFunctionType.Exp
    MUL = mybir.AluOpType.mult
    ADD = mybir.AluOpType.add

    with TileContext(nc) as tc:
        with tc.tile_pool(name="persist", bufs=1) as persist:
            qT = persist.tile([128, 2, S], BF16)
            kTd = persist.tile([128, S], BF16)
            V = persist.tile([128, HC, 65], BF16)
            aoT = persist.tile([128, 2, S], BF16)
            xt_sb = persist.tile([128, NB, HC, 512], BF16)
            wq_sb = persist.tile([128, HC, OC], BF16)
            wkv_sb = persist.tile([128, HC, 128], BF16)
            wo_sb = persist.tile([128, 2, H], BF16)
            cos_sb = persist.tile([128, S], F32)
            sin_sb = persist.tile([128, S], F32)
            emv_sb = persist.tile([128, HC], F32)
            vT = persist.tile([64, S], BF16)
            ident = persist.tile([64, 64], BF16)
            ones = persist.tile([33, 128], BF16)

            # --- input DMAs, critical-path first ---
            nc.sync.dma_start(wkv_sb[:], wkv[:])
            for c4 in range(4):
                nc.sync.dma_start(
                    xt_sb[:, 0, ts(c4, 4), :], xt[0, :, ts(c4, 4), :]
                )
            nc.gpsimd.dma_start(wq_sb[:], wq[:])
            nc.gpsimd.dma_start(cos_sb[:, 0:1024], cos_d[:, 0:1024])
            nc.gpsimd.dma_start(sin_sb[:, 0:1024], sin_d[:, 0:1024])
            nc.gpsimd.dma_start(emv_sb[:], emv_d[:])
            for b in range(1, NB):
                for c2 in range(2):
                    nc.sync.dma_start(
                        xt_sb[:, b, ts(c2, 8), :], xt[b, :, ts(c2, 8), :]
                    )
            nc.gpsimd.dma_start(cos_sb[:, 1024:2048], cos_d[:, 1024:2048])
            nc.gpsimd.dma_start(sin_sb[:, 1024:2048], sin_d[:, 1024:2048])
            nc.gpsimd.dma_start(wo_sb[:], wo[:])
            make_identity(nc, ident[:])
            nc.gpsimd.memset(ones[:], 1.0)
            for i in range(HC):
                nc.gpsimd.dma_start(V[:, i, 64:65], em_d[:, i : i + 1])

            # ---- Phase A: KV projections, K-RoPE, V transpose ----
            with (
                tc.tile_pool(name="kvps", bufs=2, space="PSUM") as kvps,
                tc.tile_pool(name="vtps", bufs=2, space="PSUM") as vtps,
                tc.tile_pool(name="q0ps", bufs=1, space="PSUM") as q0ps,
                tc.tile_pool(name="ph1t", bufs=3) as ph1t,
            ):
                # warm the EXP activation table while DMAs stream
                wrm = ph1t.tile([1, 2], F32, tag="wrm")
                nc.scalar.activation(wrm[:], ident[0:1, 0:2], EXP)
                # pre-warm the PE as soon as wkv lands: the clock needs ~3us
                # of gap-free matmuls to reach 2.4 GHz and the whole lead-in
                # otherwise runs at half clock
                for w in range(10):
                    wup = kvps.tile([128, 512], F32, tag="kv", name=f"wup_{w}")
                    nc.tensor.matmul(
                        wup[:], wkv_sb[:, w, :], wkv_sb[:, ts(w % 4, 4), :],
                        start=True, stop=True,
                    )
                for b in range(NB):
                    sb = ts(b, 512)
                    pkv = kvps.tile([128, 512], F32, tag="kv", name=f"pkv_{b}")
                    for c in range(HC):
                        nc.tensor.matmul(
                            pkv[:], wkv_sb[:, c, :], xt_sb[:, b, c, :],
                            start=c == 0, stop=c == HC - 1,
                        )
                    # K path (DVE): RoPE, duplicated on both halves
                    rotk = ph1t.tile([64, 512], F32, tag="rotk")
                    nc.vector.tensor_copy(rotk[0:32, :], pkv[32:64, :])
                    nc.vector.tensor_copy(rotk[32:64, :], pkv[0:32, :])
                    kc = ph1t.tile([64, 512], F32, tag="kc")
                    nc.vector.tensor_tensor(kc[:], pkv[0:64, :], cos_sb[0:64, sb], MUL)
                    ks = ph1t.tile([64, 512], F32, tag="ks")
                    nc.vector.tensor_tensor(ks[:], rotk[:], sin_sb[0:64, sb], MUL)
                    nc.vector.tensor_tensor(kTd[0:64, sb], kc[:], ks[:], ADD)
                    nc.vector.tensor_tensor(kTd[64:128, sb], kc[:], ks[:], ADD)
                    # V row extract on the idle Scalar engine
                    nc.scalar.copy(vT[:, sb], pkv[64:128, :])
                    if b == 0:
                        # Q(0) rides right behind KV(0): xt(0) is resident and
                        # later xt blocks are still in flight on DMA
                        pq0 = q0ps.tile([128, 512], F32, tag="q0a", name="pq0_0")
                        pq1 = q0ps.tile([128, 512], F32, tag="q0b", name="pq1_0")
                        for c in range(HC):
                            nc.tensor.matmul(
                                pq0[:], wq_sb[:, c, 0:128], xt_sb[:, 0, c, :],
                                start=c == 0, stop=c == HC - 1,
                            )
                            nc.tensor.matmul(
                                pq1[:], wq_sb[:, c, 128:256], xt_sb[:, 0, c, :],
                                start=c == 0, stop=c == HC - 1,
                            )
                # V transposes after the KV matmul stream (no mid-stream PE
                # stalls); scales on Scalar
                for i in range(HC):
                    pt = vtps.tile([128, 64], BF16, tag="vt", name=f"pt_{i}")
                    nc.tensor.transpose(pt[:], vT[:, ts(i, 128)], ident[:])
                    nc.scalar.mul(V[:, i, 0:64], pt[:], emv_sb[:, i : i + 1])
                # RoPE for Q(0)
                for p_, pq in ((0, pq0), (1, pq1)):
                    rot = ph1t.tile([128, 512], F32, tag="rot")
                    nc.vector.tensor_copy(rot[0:32, :], pq[32:64, :])
                    nc.vector.tensor_copy(rot[32:64, :], pq[0:32, :])
                    nc.vector.tensor_copy(rot[64:96, :], pq[96:128, :])
                    nc.vector.tensor_copy(rot[96:128, :], pq[64:96, :])
                    qc = ph1t.tile([128, 512], F32, tag="qc")
                    nc.vector.tensor_tensor(qc[:], pq[:], cos_sb[:, 0:512], MUL)
                    qs = ph1t.tile([128, 512], F32, tag="qs")
                    nc.vector.tensor_tensor(qs[:], rot[:], sin_sb[:, 0:512], MUL)
                    nc.vector.tensor_tensor(qT[:, p_, 0:512], qc[:], qs[:], ADD)

            # ---- main loop: Q(0), then per j: attention with Q(j+1)/Wo(j-1)
            # matmuls interleaved into the chunk slots ----
            with (
                tc.tile_pool(name="expp", bufs=4) as expp,
                tc.tile_pool(name="ropet", bufs=3) as ropet,
                tc.tile_pool(name="avp", bufs=2) as avp,
                tc.tile_pool(name="dnp", bufs=2) as dnp,
                tc.tile_pool(name="cbp", bufs=4) as cbp,
                tc.tile_pool(name="obp", bufs=3) as obp,
                tc.tile_pool(name="oap", bufs=4) as oap,
                tc.tile_pool(name="csd", bufs=4, space="DRAM") as csd,
                tc.tile_pool(name="pssc", bufs=2, space="PSUM") as pssc,
                tc.tile_pool(name="psav", bufs=1, space="PSUM") as psav,
                tc.tile_pool(name="pwork", bufs=1, space="PSUM") as pwork,
            ):

                def q_rope(jq_, pq0_, pq1_):
                    sb = ts(jq_, 512)
                    for p_, pq in ((0, pq0_), (1, pq1_)):
                        rot = ropet.tile([128, 512], F32, tag="rot")
                        nc.vector.tensor_copy(rot[0:32, :], pq[32:64, :])
                        nc.vector.tensor_copy(rot[32:64, :], pq[0:32, :])
                        nc.vector.tensor_copy(rot[64:96, :], pq[96:128, :])
                        nc.vector.tensor_copy(rot[96:128, :], pq[64:96, :])
                        qc = ropet.tile([128, 512], F32, tag="qc")
                        nc.vector.tensor_tensor(qc[:], pq[:], cos_sb[:, sb], MUL)
                        qs = ropet.tile([128, 512], F32, tag="qs")
                        nc.vector.tensor_tensor(qs[:], rot[:], sin_sb[:, sb], MUL)
                        nc.vector.tensor_tensor(qT[:, p_, sb], qc[:], qs[:], ADD)

                ob_state = {}
                oa_tiles = {}

                def emit_wo_piece(jwo, i):
                    # i in 0..15 -> (local q-chunk, hidden block)
                    jq = 4 * jwo + i // 4
                    hb = i % 4
                    tag = "wa" if i % 2 == 0 else "wb"
                    po = pwork.tile([128, 512], F32, tag=tag, name=f"po_{jq}_{hb}")
                    nc.tensor.matmul(
                        po[:], aoT[:, 0, ts(jq, 128)], wo_sb[:, 0, ts(hb, 512)],
                        start=True, stop=False,
                    )
                    nc.tensor.matmul(
                        po[:], aoT[:, 1, ts(jq, 128)], wo_sb[:, 1, ts(hb, 512)],
                        start=False, stop=True,
                    )
                    if hb == 0:
                        ob_state["ob"] = obp.tile(
                            [128, H], BF16, tag="ob", name=f"ob_{jq}"
                        )
                    ob = ob_state["ob"]
                    nc.vector.tensor_copy(ob[:, ts(hb, 512)], po[:])
                    if hb == 3:
                        eng = nc.sync if jq % 2 == 0 else nc.gpsimd
                        eng.dma_start(outp[ts(jq, 128), :], ob[:])

                def emit_wo_half_a(i):
                    # Wo(NB-1) first-contract-half: only needs aoT[:,0] (p0,
                    # normalized long ago); staged so the second half +
                    # epilogue is all that remains after the last normalize.
                    jq = 4 * (NB - 1) + i // 4
                    hb = i % 4
                    tag = "wa" if i % 2 == 0 else "wb"
                    po = pwork.tile([128, 512], F32, tag=tag, name=f"poa_{jq}_{hb}")
                    nc.tensor.matmul(
                        po[:], aoT[:, 0, ts(jq, 128)], wo_sb[:, 0, ts(hb, 512)],
                        start=True, stop=True,
                    )
                    if hb == 0:
                        oa_tiles[jq] = oap.tile(
                            [128, H], BF16, tag="oa", name=f"oa_{jq}"
                        )
                    nc.vector.tensor_copy(oa_tiles[jq][:, ts(hb, 512)], po[:])

                for j in range(NB):
                    jb = ts(j, 512)
                    for p in range(2):
                        do_q = p == 0 and j + 1 < NB
                        do_wo = (p == 1 and 1 <= j < NB - 1) or (
                            p == 0 and j == NB - 1
                        )
                        do_woa = p == 1 and j == NB - 1
                        if do_q:
                            npq0 = pwork.tile(
                                [128, 512], F32, tag="wa", name=f"pq0_{j + 1}"
                            )
                            npq1 = pwork.tile(
                                [128, 512], F32, tag="wb", name=f"pq1_{j + 1}"
                            )
                        pA = psav.tile([65, 512], F32, tag="avA", name=f"pA_{j}_{p}")
                        pB = psav.tile([65, 512], F32, tag="avB", name=f"pB_{j}_{p}")

                        def av_and_slot(i, e2):
                            st, sp = i == 0, i == HC - 1
                            nc.tensor.matmul(
                                pA[:], V[:, i, :], e2[:, 0:512], start=st, stop=sp
                            )
                            nc.tensor.matmul(
                                pB[:], V[:, i, :], e2[:, 512:1024], start=st, stop=sp
                            )
                            if do_q:
                                nc.tensor.matmul(
                                    npq0[:], wq_sb[:, i, 0:128],
                                    xt_sb[:, j + 1, i, :], start=st, stop=sp,
                                )
                                nc.tensor.matmul(
                                    npq1[:], wq_sb[:, i, 128:256],
                                    xt_sb[:, j + 1, i, :], start=st, stop=sp,
                                )
                            elif do_wo:
                                emit_wo_piece(j - 1, i)
                            elif do_woa:
                                emit_wo_half_a(i)

                        pend = []
                        for i in range(HC):
                            psAB = pssc.tile(
                                [128, 1024], F32, tag="sAB", name=f"sAB_{j}_{p}_{i}"
                            )
                            nc.tensor.matmul(
                                psAB[:, 0:512], kTd[0:64, ts(i, 128)],
                                qT[0:64, p, jb], start=True, stop=True,
                            )
                            nc.tensor.matmul(
                                psAB[:, 512:1024], kTd[64:128, ts(i, 128)],
                                qT[64:128, p, jb], start=True, stop=True,
                            )
                            e2 = expp.tile(
                                [128, 1024], BF16, tag="e2", name=f"e2_{j}_{p}_{i}"
                            )
                            nc.scalar.activation(e2[:], psAB[:], EXP, scale=s_qk)
                            pend.append((i, e2))
                            if len(pend) > 2:
                                av_and_slot(*pend.pop(0))
                        for it in pend:
                            av_and_slot(*it)

                        # stage AV out of PSUM, build denominators, normalize
                        av = avp.tile([128, 512], F32, tag="av", name=f"av_{j}_{p}")
                        nc.vector.tensor_copy(av[0:64, :], pA[0:64, :])
                        nc.vector.tensor_copy(av[64:128, :], pB[0:64, :])
                        dn = dnp.tile([33, 512], F32, tag="dn")
                        nc.vector.tensor_copy(dn[0:1, :], pA[64:65, :])
                        nc.vector.tensor_copy(dn[32:33, :], pB[64:65, :])
                        rcs = dnp.tile([33, 512], F32, tag="rcs")
                        scr = dnp.tile([33, 512], F32, tag="scr")
                        nc.vector.reciprocal_approx_accurate(rcs[:], dn[:], scr[:])
                        if j == NB - 1 and p == 1:
                            # no DRAM round trip at the tail: broadcast the
                            # reciprocals via rank-1 matmuls into a spare
                            # scores bank, normalize per q-chunk so the Wo
                            # epilogue starts immediately
                            rcs_b = dnp.tile([33, 512], BF16, tag="rcsb")
                            nc.scalar.copy(rcs_b[0:1, :], rcs[0:1, :])
                            nc.scalar.copy(rcs_b[32:33, :], rcs[32:33, :])
                            cb_ps = pssc.tile(
                                [128, 1024], F32, tag="sAB", name="cb_ps"
                            )
                            nc.tensor.matmul(
                                cb_ps[0:64, 0:512], ones[0:1, 0:64],
                                rcs_b[0:1, :], start=True, stop=True,
                            )
                            nc.tensor.matmul(
                                cb_ps[64:128, 0:512], ones[32:33, 0:64],
                                rcs_b[32:33, :], start=True, stop=True,
                            )
                            for qc in range(4):
                                qs_ = slice(qc * 128, (qc + 1) * 128)
                                nc.vector.tensor_tensor(
                                    aoT[:, p, 1536 + 128 * qc : 1664 + 128 * qc],
                                    av[:, qs_], cb_ps[:, qs_], MUL,
                                )
                        else:
                            cs_dram = csd.tile([2, 1, 512], F32, tag="csd")
                            nc.sync.dma_start(cs_dram[0], rcs[0:1, :])
                            nc.sync.dma_start(cs_dram[1], rcs[32:33, :])
                            cb = cbp.tile(
                                [128, 512], F32, tag="cb", name=f"cb_{j}_{p}"
                            )
                            for h in range(2):
                                nc.sync.dma_start(
                                    cb[h * 64 : (h + 1) * 64, :],
                                    cs_dram[h].to_broadcast((64, 512)),
                                )
                            nc.vector.tensor_tensor(aoT[:, p, jb], av[:], cb[:], MUL)

                        if do_q:
                            q_rope(j + 1, npq0, npq1)

                # Wo(NB-1) epilogue: second contract half + add the staged
                # first half, then the last output DMAs. Warm matmuls first so
                # the PE re-ramps while the last normalize chain drains.
                for w in range(8):
                    wrmt = pwork.tile(
                        [128, 512], F32, tag="wa" if w % 2 == 0 else "wb",
                        name=f"wrmt_{w}",
                    )
                    nc.tensor.matmul(
                        wrmt[:], wkv_sb[:, w, :], xt_sb[:, NB - 1, w, :],
                        start=True, stop=True,
                    )
                for i in range(HC):
                    jq = 4 * (NB - 1) + i // 4
                    hb = i % 4
                    tag = "wa" if i % 2 == 0 else "wb"
                    po = pwork.tile([128, 512], F32, tag=tag, name=f"pob_{jq}_{hb}")
                    nc.tensor.matmul(
                        po[:], aoT[:, 1, ts(jq, 128)], wo_sb[:, 1, ts(hb, 512)],
                        start=True, stop=True,
                    )
                    if hb == 0:
                        ob_state["ob"] = obp.tile(
                            [128, H], BF16, tag="ob", name=f"obf_{jq}"
                        )
                    ob = ob_state["ob"]
                    nc.vector.tensor_tensor(
                        ob[:, ts(hb, 512)], po[:], oa_tiles[jq][:, ts(hb, 512)], ADD
                    )
                    if hb == 3:
                        eng = nc.sync if jq % 2 == 0 else nc.gpsimd
                        eng.dma_start(outp[ts(jq, 128), :], ob[:])

    nc.compile()
    return nc


def kernel(
    hidden_states,
    attention_mask,
    position_ids,
    wq,
    wk,
    wv,
    wo,
    _trace=False,
):
    global LAST_EXEC_NS, LAST_TRACE
    x = np.asarray(hidden_states, np.float32)[0]  # [S, H]
    mask = np.asarray(attention_mask, np.float32)[0]  # [S]
    pos = np.asarray(position_ids)[0].astype(np.float32)  # [S]

    wq_t, s_q = _ternarize(wq)
    wk_t, s_k = _ternarize(wk)
    wv_t, s_v = _ternarize(wv)
    wo_t, s_o = _ternarize(wo)
    s_qk = float(np.float32(s_q) * np.float32(s_k) / np.float32(8.0))
    s_vo = np.float32(s_v) * np.float32(s_o)

    key = ("v6", s_qk)
    if key not in _CACHE:
        _CACHE.clear()
        _CACHE[key] = _build_program(s_qk)
    nc = _CACHE[key]

    # shared inputs
    xt_host = np.ascontiguousarray(
        x.T.reshape(HC, 128, NB, 512).transpose(2, 1, 0, 3)
    ).astype(bfloat16)
    inv = (
        1.0 / (10000.0 ** (np.arange(0, D, 2, dtype=np.float32) / np.float32(D)))
    ).astype(np.float32)
    fr = pos[:, None] * inv[None, :]  # [S, 32]
    emb = np.concatenate([fr, fr], axis=1)  # [S, 64]
    cos64 = np.cos(emb).astype(np.float32)
    sin64 = np.sin(emb).astype(np.float32)
    sin64[:, : D // 2] *= -1.0
    cos128 = np.ascontiguousarray(np.vstack([cos64.T, cos64.T]))  # [128, S]
    sin128 = np.ascontiguousarray(np.vstack([sin64.T, sin64.T]))
    expmask = np.exp(mask).astype(np.float32)  # [S]
    em_r = np.ascontiguousarray(expmask.reshape(HC, 128).T).astype(bfloat16)
    emv_r = np.ascontiguousarray(
        (expmask * s_vo).reshape(HC, 128).T
    ).astype(np.float32)

    in_maps = []
    for c in range(NCORES):
        wq_c = np.ascontiguousarray(
            wq_t[c * OC : (c + 1) * OC, :].T.reshape(HC, 128, OC).transpose(1, 0, 2)
        ).astype(bfloat16)
        wk_c = wk_t[c * D : (c + 1) * D, :].T  # [H, 64]
        wv_c = wv_t[c * D : (c + 1) * D, :].T
        wkv_c = np.ascontiguousarray(
            np.concatenate([wk_c, wv_c], axis=1).reshape(HC, 128, 128).transpose(1, 0, 2)
        ).astype(bfloat16)
        wo_c = np.ascontiguousarray(
            wo_t[:, c * OC : (c + 1) * OC].T.reshape(2, 128, H).transpose(1, 0, 2)
        ).astype(bfloat16)
        in_maps.append(
            {
                "xt": xt_host,
                "wq_t": wq_c,
                "wkv_t": wkv_c,
                "wo_t": wo_c,
                "cos_t": cos128,
                "sin_t": sin128,
                "emv_t": emv_r,
                "em_t": em_r,
            }
        )

    res = run_bass_kernel_spmd(
        nc, in_maps, core_ids=list(range(NCORES)), trace=bool(_trace)
    )
    LAST_EXEC_NS = res.exec_time_ns
    LAST_TRACE = res.instructions_and_trace[1] if res.instructions_and_trace else None

    out = res.results[0]["outp"].astype(np.float32)
    for c in range(1, NCORES):
        out = out + res.results[c]["outp"].astype(np.float32)
    return out.reshape(1, S, H).astype(np.float32)
